# revision 8
# baseline (speedup 1.0000x reference)
"""EventAwareAttention Trainium2 kernel.

Math (per batch element b, matching the reference):
    Q = X @ Wq.T + bq ;  K = X @ Wk.T + bk ;  V = X @ Wv.T + bv
    scores = (Q @ K.T) / sqrt(512) * shape_ratio + event_weight * event_flag
    attn = softmax(scores, axis=-1)           # event term is a per-query
    context = attn @ V                        # constant -> softmax-invariant

Sharding: pure data-parallel — batch B=8, one batch element per NeuronCore.

Per-core dataflow (all matmuls bf16 with fp32 PSUM accumulation):
    XT  = X^T  via PE transposes                         [h, s]
    variant "general" (any biases):
        QT = (Wq @ X^T + bq) * shape_ratio/sqrt(512)     [o, s]
        KT =  Wk @ X^T + bk                              [o, s]
        E  = exp(K Q^T) per [k=128, q=512] tile          [k, q]
    variant "fold" (bq == bk == 0):
        A  = (Wk^T @ Wq) * shape_ratio/sqrt(512)         [h, h]   (16 matmuls)
        GT = A^T @ X^T                                   [h, s]
        E  = exp(G X^T) = exp(K Q^T) per tile            [k, q]
    V   = X @ Wv^T  (+ bv folded in at the end)          [s, o]
    ctx PSUM[q,512] = sum_k E_chunk(stationary) @ V      rowsum via N=1 matmul
    context = ctx * (1/rowsum) + bv
    attn    = (PE-transpose of E chunks) * (1/rowsum)    [q, k]

The per-query additive event bias is dropped: softmax over k of
(s[q,k] + c[q]) is exactly softmax(s[q,k]); neither output depends on it.
`lengths` is unused by the reference.
"""

import contextlib
import math

import numpy as np

import concourse.mybir as mybir
import concourse.tile as tile
from concourse import bacc
from concourse.bass_utils import run_bass_kernel_spmd
from concourse.masks import make_identity

B, S, H = 8, 2048, 512
P = 128
HC = H // P      # 4  h chunks
ST = S // P      # 16 s tiles
QBW = 512        # q block width for the scores phase
NQB = S // QBW   # 4
F32 = mybir.dt.float32
BF16 = mybir.dt.bfloat16

_CACHE = {}


def _emit(tc, nc, aps, fold):
    X, Wq, Wk, Wv, bq_pc, bk_pc, bv_rep, sr_rep, ctx_o, attn_o = aps

    with contextlib.ExitStack() as ctx:
        const = ctx.enter_context(tc.tile_pool(name="const", bufs=1))
        work = ctx.enter_context(tc.tile_pool(name="work", bufs=3))
        epool = ctx.enter_context(tc.tile_pool(name="epool", bufs=2))
        apool = ctx.enter_context(tc.tile_pool(name="apool", bufs=4))
        ps_sc = ctx.enter_context(tc.tile_pool(name="ps_sc", bufs=2, space="PSUM"))
        ps_c = ctx.enter_context(tc.tile_pool(name="ps_c", bufs=3, space="PSUM"))
        ps_tr = ctx.enter_context(tc.tile_pool(name="ps_tr", bufs=2, space="PSUM"))
        ps_rs = ctx.enter_context(tc.tile_pool(name="ps_rs", bufs=1, space="PSUM"))

        # ---- constants / small tiles ----
        ident = const.tile([P, P], BF16, name="ident")
        make_identity(nc, ident)

        smalls = const.tile([P, 2 + 2 * HC], F32, name="smalls")
        sr_s = smalls[:, 0:1]               # shape_ratio / sqrt(512)
        bqs = smalls[:, 2:2 + HC]           # bq * srscale, [p, oc]
        bks = smalls[:, 2 + HC:2 + 2 * HC]  # bk raw, [p, oc]
        nc.sync.dma_start(smalls[:, 1:2], sr_rep[:])
        nc.vector.tensor_scalar_mul(sr_s, smalls[:, 1:2], 1.0 / math.sqrt(H))
        if not fold:
            nc.sync.dma_start(bks[:], bk_pc[:])
            nc.sync.dma_start(bqs[:], bq_pc[:])
            nc.vector.tensor_scalar_mul(bqs, bqs, sr_s)

        bv_sb = const.tile([P, H], F32, name="bv_sb")
        nc.sync.dma_start(bv_sb[:], bv_rep[:])

        ones_col = const.tile([P, 1], BF16, name="ones_col")
        nc.vector.memset(ones_col[:], 1.0)

        def load_cast(src_ap, tag):
            """DMA a [128, 512] f32 block and cast to bf16."""
            st_ = work.tile([P, H], F32, tag="fstage", name="fstage")
            nc.sync.dma_start(st_[:], src_ap)
            bf = work.tile([P, H], BF16, tag=tag, name=tag)
            nc.vector.tensor_copy(bf[:], st_[:])
            return bf

        def transpose4_to(dst_ap, src_bf):
            """PE-transpose the four 128x128 chunks of src [128,512] into
            dst AP of shape [128, 4, 128] via one psum tile + one copy."""
            pt4 = ps_tr.tile([P, HC, P], BF16, tag="tr", name="pt4")
            for j in range(HC):
                nc.tensor.transpose(pt4[:, j, :], src_bf[:, j * P:(j + 1) * P],
                                    ident[:])
            nc.vector.tensor_copy(dst_ap, pt4[:])

        # ---- weights first (smaller DMAs -> PE starts sooner) ----
        WTv = const.tile([P, HC, H], BF16, name="WTv")
        for oc in range(HC):
            wbf = load_cast(Wv[oc * P:(oc + 1) * P, :], "wbf")
            transpose4_to(WTv[:, :, oc * P:(oc + 1) * P], wbf)

        if fold:
            # A = (Wk^T @ Wq) * srscale   [hi, ho] with hi on partitions
            wqb = [load_cast(Wq[oc * P:(oc + 1) * P, :], "wqb%d" % oc)
                   for oc in range(HC)]
            wkb = [load_cast(Wk[oc * P:(oc + 1) * P, :], "wkb%d" % oc)
                   for oc in range(HC)]
            A_sb = const.tile([P, HC, H], BF16, name="A_sb")
            for hic in range(HC):
                pa = ps_sc.tile([P, H], F32, tag="sc", name="pa")
                for oc in range(HC):
                    nc.tensor.matmul(
                        pa[:], wkb[oc][:, hic * P:(hic + 1) * P], wqb[oc][:],
                        start=(oc == 0), stop=(oc == HC - 1))
                nc.vector.tensor_scalar_mul(A_sb[:, hic, :], pa[:], sr_s)

        # ---- X^T ----
        XT = const.tile([P, HC, S], BF16, name="XT")
        for st in range(ST):
            xbf = load_cast(X[st * P:(st + 1) * P, :], "xbf")
            transpose4_to(XT[:, :, st * P:(st + 1) * P], xbf)

        if fold:
            # GT = A^T @ X^T   [ho, s]
            GT = const.tile([P, HC, S], BF16, name="GT")
            for hoc in range(HC):
                for sb in range(NQB):
                    sl = slice(sb * 512, (sb + 1) * 512)
                    pg = ps_sc.tile([P, 512], F32, tag="sc", name="pg")
                    for hic in range(HC):
                        nc.tensor.matmul(
                            pg[:], A_sb[:, hic, hoc * P:(hoc + 1) * P],
                            XT[:, hic, sl],
                            start=(hic == 0), stop=(hic == HC - 1))
                    nc.vector.tensor_copy(GT[:, hoc, sl], pg[:])
            SL = GT      # scores lhsT source [c, k]
            SR = XT      # scores rhs source [c, q]
        else:
            WTq = const.tile([P, HC, H], BF16, name="WTq")
            WTk = const.tile([P, HC, H], BF16, name="WTk")
            for W_, WT in ((Wq, WTq), (Wk, WTk)):
                for oc in range(HC):
                    wbf = load_cast(W_[oc * P:(oc + 1) * P, :], "wbf")
                    transpose4_to(WT[:, :, oc * P:(oc + 1) * P], wbf)
            QT = const.tile([P, HC, S], BF16, name="QT")
            KT = const.tile([P, HC, S], BF16, name="KT")
            for oc in range(HC):
                for sb in range(NQB):
                    sl = slice(sb * 512, (sb + 1) * 512)
                    pq = ps_sc.tile([P, 512], F32, tag="sc", name="pq")
                    for hc in range(HC):
                        nc.tensor.matmul(
                            pq[:], WTq[:, hc, oc * P:(oc + 1) * P],
                            XT[:, hc, sl],
                            start=(hc == 0), stop=(hc == HC - 1))
                    # QT = Qraw*srscale + bq*srscale
                    nc.vector.tensor_scalar(
                        QT[:, oc, sl], pq[:], sr_s, bqs[:, oc:oc + 1],
                        mybir.AluOpType.mult, mybir.AluOpType.add)
                    pk = ps_sc.tile([P, 512], F32, tag="sc", name="pk")
                    for hc in range(HC):
                        nc.tensor.matmul(
                            pk[:], WTk[:, hc, oc * P:(oc + 1) * P],
                            XT[:, hc, sl],
                            start=(hc == 0), stop=(hc == HC - 1))
                    nc.vector.tensor_scalar_add(KT[:, oc, sl], pk[:],
                                                bks[:, oc:oc + 1])
            SL = KT
            SR = QT

        V_sb = const.tile([P, ST, 512], BF16, name="V_sb")
        for st in range(ST):
            pv = ps_sc.tile([P, 512], F32, tag="sc", name="pv")
            for hc in range(HC):
                nc.tensor.matmul(
                    pv[:], XT[:, hc, st * P:(st + 1) * P], WTv[:, hc, :],
                    start=(hc == 0), stop=(hc == HC - 1))
            nc.vector.tensor_copy(V_sb[:, st, :], pv[:])

        # ---- attention main loop over q blocks of 512 ----
        # software-pipelined: scores/exp for block qb+1 are emitted before
        # context/attn of block qb so PE never stalls on ACT's exp output.
        E_tiles = {}

        def emit_scores(qb):
            qsl = slice(qb * QBW, (qb + 1) * QBW)
            E = epool.tile([P, ST, QBW], BF16, tag="E", name="E")
            E_tiles[qb] = E
            for kt in range(ST):
                ps = ps_sc.tile([P, QBW], F32, tag="sc", name="ps")
                for c in range(HC):
                    nc.tensor.matmul(
                        ps[:], SL[:, c, kt * P:(kt + 1) * P], SR[:, c, qsl],
                        start=(c == 0), stop=(c == HC - 1))
                nc.scalar.activation(E[:, kt, :], ps[:],
                                     mybir.ActivationFunctionType.Exp)

        def emit_ctx_attn(qb):
            E = E_tiles.pop(qb)
            recips = work.tile([P, 4], F32, tag="recips", name="recips")
            for qs in range(4):
                qt = qb * 4 + qs
                esl = slice(qs * P, (qs + 1) * P)
                pc = ps_c.tile([P, 512], F32, tag="ctx", name="pc")
                pr = ps_rs.tile([P, 1], F32, tag="rs", name="pr")
                for kt in range(ST):
                    nc.tensor.matmul(
                        pc[:], E[:, kt, esl], V_sb[:, kt, :],
                        start=(kt == 0), stop=(kt == ST - 1))
                    nc.tensor.matmul(
                        pr[:], E[:, kt, esl], ones_col[:],
                        start=(kt == 0), stop=(kt == ST - 1))
                nc.vector.reciprocal(recips[:, qs:qs + 1], pr[:])
                cs = work.tile([P, 512], F32, tag="ctxsb", name="cs")
                nc.vector.tensor_scalar_mul(cs[:], pc[:], recips[:, qs:qs + 1])
                nc.vector.tensor_tensor(cs[:], cs[:], bv_sb[:],
                                        mybir.AluOpType.add)
                nc.sync.dma_start(ctx_o[qt * P:(qt + 1) * P, :], cs[:])

                at = apool.tile([P, S], F32, tag="arow", name="at")
                for kg in range(ST // HC):
                    pt4 = ps_tr.tile([P, HC, P], BF16, tag="tr", name="pt4a")
                    for j in range(HC):
                        kt = kg * HC + j
                        nc.tensor.transpose(pt4[:, j, :], E[:, kt, esl],
                                            ident[:])
                    nc.scalar.mul(at[:, kg * 512:(kg + 1) * 512],
                                  pt4[:].rearrange("p a b -> p (a b)"),
                                  recips[:, qs:qs + 1])
                nc.sync.dma_start(attn_o[qt * P:(qt + 1) * P, :], at[:])

        emit_scores(0)
        for qb in range(NQB):
            if qb + 1 < NQB:
                emit_scores(qb + 1)
            emit_ctx_attn(qb)


def build_nc(fold):
    nc = bacc.Bacc("TRN2", target_bir_lowering=False, debug=False)
    X = nc.dram_tensor("X", [S, H], F32, kind="ExternalInput").ap()
    Wq = nc.dram_tensor("Wq", [H, H], F32, kind="ExternalInput").ap()
    Wk = nc.dram_tensor("Wk", [H, H], F32, kind="ExternalInput").ap()
    Wv = nc.dram_tensor("Wv", [H, H], F32, kind="ExternalInput").ap()
    bq_pc = nc.dram_tensor("bq_pc", [P, HC], F32, kind="ExternalInput").ap()
    bk_pc = nc.dram_tensor("bk_pc", [P, HC], F32, kind="ExternalInput").ap()
    bv_rep = nc.dram_tensor("bv_rep", [P, H], F32, kind="ExternalInput").ap()
    sr_rep = nc.dram_tensor("sr_rep", [P, 1], F32, kind="ExternalInput").ap()
    ctx_o = nc.dram_tensor("context", [S, H], F32, kind="ExternalOutput").ap()
    attn_o = nc.dram_tensor("attn", [S, S], F32, kind="ExternalOutput").ap()
    with tile.TileContext(nc) as tc:
        _emit(tc, nc, (X, Wq, Wk, Wv, bq_pc, bk_pc, bv_rep, sr_rep,
                       ctx_o, attn_o), fold)
    nc.compile()
    return nc


def get_nc(fold=True):
    key = "fold" if fold else "general"
    if key not in _CACHE:
        _CACHE[key] = build_nc(fold)
    return _CACHE[key]


def make_in_maps(inputs):
    X = np.ascontiguousarray(np.asarray(inputs["X"], dtype=np.float32))
    Wq = np.ascontiguousarray(np.asarray(inputs["Wq"], dtype=np.float32))
    Wk = np.ascontiguousarray(np.asarray(inputs["Wk"], dtype=np.float32))
    Wv = np.ascontiguousarray(np.asarray(inputs["Wv"], dtype=np.float32))
    bq = np.asarray(inputs["bq"], dtype=np.float32)
    bk = np.asarray(inputs["bk"], dtype=np.float32)
    bv = np.asarray(inputs["bv"], dtype=np.float32)
    sr = float(np.asarray(inputs["shape_ratio"]).reshape(-1)[0])
    # input marshalling: reshape/replicate only (no tensor math)
    bq_pc = np.ascontiguousarray(bq.reshape(HC, P).T)
    bk_pc = np.ascontiguousarray(bk.reshape(HC, P).T)
    bv_rep = np.ascontiguousarray(np.broadcast_to(bv.reshape(1, H), (P, H)))
    sr_rep = np.full((P, 1), sr, dtype=np.float32)
    fold = not (np.any(bq) or np.any(bk))
    return [
        dict(X=X[b], Wq=Wq, Wk=Wk, Wv=Wv, bq_pc=bq_pc, bk_pc=bk_pc,
             bv_rep=bv_rep, sr_rep=sr_rep)
        for b in range(B)
    ], fold


def kernel(**inputs):
    in_maps, fold = make_in_maps(inputs)
    nc = get_nc(fold)
    r = run_bass_kernel_spmd(nc, in_maps, list(range(B)))
    context = np.stack([r.results[b]["context"] for b in range(B)])
    attn = np.stack([r.results[b]["attn"] for b in range(B)])
    return context, attn


# revision 20
# speedup vs baseline: 1.0087x; 1.0087x over previous
"""EventAwareAttention Trainium2 kernel.

Math (per batch element b, matching the reference):
    Q = X @ Wq.T + bq ;  K = X @ Wk.T + bk ;  V = X @ Wv.T + bv
    scores = (Q @ K.T) / sqrt(512) * shape_ratio + event_weight * event_flag
    attn = softmax(scores, axis=-1)           # event term is a per-query
    context = attn @ V                        # constant -> softmax-invariant

Sharding: pure data-parallel — batch B=8, one batch element per NeuronCore.

Per-core dataflow (all matmuls bf16 with fp32 PSUM accumulation):
    XT  = X^T  via PE transposes                         [h, s]
    variant "general" (any biases):
        QT = (Wq @ X^T + bq) * shape_ratio/sqrt(512)     [o, s]
        KT =  Wk @ X^T + bk                              [o, s]
        E  = exp(K Q^T) per [k=128, q=512] tile          [k, q]
    variant "fold" (bq == bk == 0):
        A  = (Wk^T @ Wq) * shape_ratio/sqrt(512)         [h, h]   (16 matmuls)
        GT = A^T @ X^T                                   [h, s]
        E  = exp(G X^T) = exp(K Q^T) per tile            [k, q]
    V   = X @ Wv^T  (+ bv folded in at the end)          [s, o]
    ctx PSUM[q,512] = sum_k E_chunk(stationary) @ V      rowsum via N=1 matmul
    context = ctx * (1/rowsum) + bv
    attn    = (PE-transpose of E chunks) * (1/rowsum)    [q, k]

The per-query additive event bias is dropped: softmax over k of
(s[q,k] + c[q]) is exactly softmax(s[q,k]); neither output depends on it.
`lengths` is unused by the reference.
"""

import contextlib
import math

import numpy as np

import concourse.mybir as mybir
import concourse.tile as tile
from concourse import bacc
from concourse.bass_utils import run_bass_kernel_spmd
from concourse.masks import make_identity

B, S, H = 8, 2048, 512
P = 128
HC = H // P      # 4  h chunks
ST = S // P      # 16 s tiles
QBW = 512        # q block width for the scores phase
NQB = S // QBW   # 4
F32 = mybir.dt.float32
BF16 = mybir.dt.bfloat16

_CACHE = {}


def _emit(tc, nc, aps, fold):
    X, Wq, Wk, Wv, bq_pc, bk_pc, bv_rep, sr_rep, ctx_o, attn_o = aps

    with contextlib.ExitStack() as ctx:
        const = ctx.enter_context(tc.tile_pool(name="const", bufs=1))
        work = ctx.enter_context(tc.tile_pool(name="work", bufs=3))
        epool = ctx.enter_context(tc.tile_pool(name="epool", bufs=2))
        apool = ctx.enter_context(tc.tile_pool(name="apool", bufs=4 if fold else 3))
        ps_sc = ctx.enter_context(tc.tile_pool(name="ps_sc", bufs=2, space="PSUM"))
        ps_c = ctx.enter_context(tc.tile_pool(name="ps_c", bufs=2, space="PSUM"))
        ps_tr = ctx.enter_context(tc.tile_pool(name="ps_tr", bufs=2, space="PSUM"))
        ps_rs = ctx.enter_context(tc.tile_pool(name="ps_rs", bufs=1, space="PSUM"))

        # ---- constants / small tiles ----
        ident = const.tile([P, P], BF16, name="ident")
        make_identity(nc, ident)

        def transpose4_to(dst_ap, src_bf):
            """PE-transpose the four 128x128 chunks of src [128,512] into
            dst AP of shape [128, 4, 128] via one psum tile + one copy."""
            pt4 = ps_tr.tile([P, HC, P], BF16, tag="tr", name="pt4")
            for j in range(HC):
                nc.tensor.transpose(pt4[:, j, :], src_bf[:, j * P:(j + 1) * P],
                                    ident[:])
            nc.vector.tensor_copy(dst_ap, pt4[:])

        XT = const.tile([P, HC, S], BF16, name="XT")
        V_sb = const.tile([P, ST, 512], BF16, name="V_sb")
        bv_sb = const.tile([P, H], F32, name="bv_sb")
        Xv = X.rearrange("(st p) h -> p st h", p=P)

        def load_x_group(g, split=False, emit_v=False):
            xg = work.tile([P, 4, H], F32, tag="xg", name="xg", bufs=3)
            if split:
                for j in range(4):
                    nc.sync.dma_start(xg[:, j, :], Xv[:, g * 4 + j, :])
            else:
                nc.sync.dma_start(xg[:], Xv[:, g * 4:(g + 1) * 4, :])
            for j in range(4):
                st = g * 4 + j
                xbf = work.tile([P, H], BF16, tag="xbf", name="xbf")
                nc.vector.tensor_copy(xbf[:], xg[:, j, :])
                transpose4_to(XT[:, :, st * P:(st + 1) * P], xbf)
                if emit_v:
                    emit_vproj(st)

        def emit_vproj(st):
            pv = ps_sc.tile([P, 512], F32, tag="sc", name="pv")
            for hc in range(HC):
                nc.tensor.matmul(
                    pv[:], XT[:, hc, st * P:(st + 1) * P], WTv[:, hc, :],
                    start=(hc == 0), stop=(hc == HC - 1))
            nc.vector.tensor_copy(V_sb[:, st, :], pv[:])

        load_x_group(0, split=True)

        smalls = const.tile([P, 2 + 2 * HC], F32, name="smalls")
        sr_s = smalls[:, 0:1]               # shape_ratio / sqrt(512)
        bqs = smalls[:, 2:2 + HC]           # bq * srscale, [p, oc]
        bks = smalls[:, 2 + HC:2 + 2 * HC]  # bk raw, [p, oc]
        nc.sync.dma_start(smalls[:, 1:2], sr_rep[:])
        nc.vector.tensor_scalar_mul(sr_s, smalls[:, 1:2], 1.0 / math.sqrt(H))
        if not fold:
            nc.sync.dma_start(bks[:], bk_pc[:])
            nc.sync.dma_start(bqs[:], bq_pc[:])
            nc.vector.tensor_scalar_mul(bqs, bqs, sr_s)

        ones_col = const.tile([P, 1], BF16, name="ones_col")
        nc.vector.memset(ones_col[:], 1.0)
        one_1x1 = const.tile([1, 1], F32, name="one_1x1")
        nc.vector.memset(one_1x1[:], 1.0)

        # ---- weights (one DMA per matrix) ----
        def load_w(Wx, tag):
            wst = work.tile([P, HC, H], F32, tag="wstage", name="wstage", bufs=2)
            nc.sync.dma_start(wst[:], Wx.rearrange("(oc p) h -> p oc h", p=P))
            wbf = work.tile([P, HC, H], BF16, tag=tag, name=tag, bufs=1)
            for oc in range(HC):
                nc.scalar.copy(wbf[:, oc, :], wst[:, oc, :])
            return wbf

        WTv = const.tile([P, HC, H], BF16, name="WTv")
        wvb = load_w(Wv, "wvb")
        for oc in range(HC):
            transpose4_to(WTv[:, :, oc * P:(oc + 1) * P], wvb[:, oc, :])

        if fold:
            # A = (Wk^T @ Wq) * srscale   [hi, ho] with hi on partitions
            wqb_t = load_w(Wq, "wqb")
            wkb_t = load_w(Wk, "wkb")
            wqb = [wqb_t[:, oc, :] for oc in range(HC)]
            wkb = [wkb_t[:, oc, :] for oc in range(HC)]
            A_sb = const.tile([P, HC, H], BF16, name="A_sb")
            for hic in range(HC):
                pa = ps_sc.tile([P, H], F32, tag="sc", name="pa")
                for oc in range(HC):
                    nc.tensor.matmul(
                        pa[:], wkb[oc][:, hic * P:(hic + 1) * P], wqb[oc][:],
                        start=(oc == 0), stop=(oc == HC - 1))
                nc.scalar.mul(A_sb[:, hic, :], pa[:], sr_s)


        def emit_gt_block(GT, sb):
            for hoc in range(HC):
                sl = slice(sb * 512, (sb + 1) * 512)
                pg = ps_sc.tile([P, 512], F32, tag="sc", name="pg")
                for hic in range(HC):
                    nc.tensor.matmul(
                        pg[:], A_sb[:, hic, hoc * P:(hoc + 1) * P],
                        XT[:, hic, sl],
                        start=(hic == 0), stop=(hic == HC - 1))
                nc.scalar.copy(GT[:, hoc, sl], pg[:])

        if fold:
            # GT = A^T @ X^T   [ho, s]
            GT = const.tile([P, HC, S], BF16, name="GT")
            for g in range(1, 4):
                load_x_group(g)
            nc.sync.dma_start(bv_sb[:], bv_rep[:])
            for st in range(ST):
                emit_vproj(st)
            for sb in range(NQB):
                emit_gt_block(GT, sb)
            SL = GT      # scores lhsT source [c, k]
            SR = XT      # scores rhs source [c, q]
        else:
            WTq = const.tile([P, HC, H], BF16, name="WTq")
            WTk = const.tile([P, HC, H], BF16, name="WTk")
            for W_, WT, tg in ((Wq, WTq, "wqb"), (Wk, WTk, "wkb")):
                wb = load_w(W_, tg)
                for oc in range(HC):
                    transpose4_to(WT[:, :, oc * P:(oc + 1) * P], wb[:, oc, :])
            for g in range(1, 4):
                load_x_group(g)
            nc.sync.dma_start(bv_sb[:], bv_rep[:])
            for st in range(ST):
                emit_vproj(st)
            QT = const.tile([P, HC, S], BF16, name="QT")
            KT = const.tile([P, HC, S], BF16, name="KT")
            for oc in range(HC):
                for sb in range(NQB):
                    sl = slice(sb * 512, (sb + 1) * 512)
                    pq = ps_sc.tile([P, 512], F32, tag="sc", name="pq")
                    for hc in range(HC):
                        nc.tensor.matmul(
                            pq[:], WTq[:, hc, oc * P:(oc + 1) * P],
                            XT[:, hc, sl],
                            start=(hc == 0), stop=(hc == HC - 1))
                    # QT = Qraw*srscale + bq*srscale
                    nc.vector.tensor_scalar(
                        QT[:, oc, sl], pq[:], sr_s, bqs[:, oc:oc + 1],
                        mybir.AluOpType.mult, mybir.AluOpType.add)
                    pk = ps_sc.tile([P, 512], F32, tag="sc", name="pk")
                    for hc in range(HC):
                        nc.tensor.matmul(
                            pk[:], WTk[:, hc, oc * P:(oc + 1) * P],
                            XT[:, hc, sl],
                            start=(hc == 0), stop=(hc == HC - 1))
                    nc.vector.tensor_scalar_add(KT[:, oc, sl], pk[:],
                                                bks[:, oc:oc + 1])
            SL = KT
            SR = QT

        # ---- attention main loop over q blocks of 512 ----
        # software-pipelined: scores/exp for block qb+1 are emitted before
        # context/attn of block qb so PE never stalls on ACT's exp output.
        E_tiles = {}
        R_tiles = {}

        def emit_scores(qb):
            qsl = slice(qb * QBW, (qb + 1) * QBW)
            E = epool.tile([P, ST, QBW], BF16, tag="E", name="E")
            E_tiles[qb] = E
            for kt in range(ST):
                ps = ps_sc.tile([P, QBW], F32, tag="sc", name="ps")
                for c in range(HC):
                    nc.tensor.matmul(
                        ps[:], SL[:, c, kt * P:(kt + 1) * P], SR[:, c, qsl],
                        start=(c == 0), stop=(c == HC - 1))
                nc.scalar.activation(E[:, kt, :], ps[:],
                                     mybir.ActivationFunctionType.Exp)
            # rowsum over k as an M=1 matmul row (ones stationary: trivial
            # weight load), then 4 tiny f32 transposes -> per-partition recips
            rs_row = ps_rs.tile([1, QBW], F32, tag="rs", name="rs_row")
            for kt in range(ST):
                nc.tensor.matmul(rs_row[:], ones_col[:], E[:, kt, :],
                                 start=(kt == 0), stop=(kt == ST - 1))
            rs_sb = work.tile([1, QBW], F32, tag="rssb", name="rs_sb", bufs=1)
            nc.vector.tensor_copy(rs_sb[:], rs_row[:])
            rst = ps_rs.tile([P, 4], F32, tag="rst", name="rst", bufs=1)
            for qs in range(4):
                nc.tensor.transpose(rst[:, qs:qs + 1],
                                    rs_sb[0:1, qs * P:(qs + 1) * P],
                                    one_1x1[:])
            recips = work.tile([P, 4], F32, tag="recips", name="recips")
            nc.vector.reciprocal(recips[:], rst[:])
            R_tiles[qb] = recips

        def emit_ctx_attn(qb):
            E = E_tiles.pop(qb)
            recips = R_tiles.pop(qb)
            cs4 = work.tile([P, 4, 512], F32, tag="ctxsb", name="cs4", bufs=2)
            for qs in range(4):
                qt = qb * 4 + qs
                esl = slice(qs * P, (qs + 1) * P)
                pc = ps_c.tile([P, 512], F32, tag="ctx", name="pc")
                for kt in range(ST):
                    nc.tensor.matmul(
                        pc[:], E[:, kt, esl], V_sb[:, kt, :],
                        start=(kt == 0), stop=(kt == ST - 1))
                nc.vector.tensor_scalar_mul(cs4[:, qs, :], pc[:],
                                            recips[:, qs:qs + 1])
                nc.vector.tensor_tensor(cs4[:, qs, :], cs4[:, qs, :], bv_sb[:],
                                        mybir.AluOpType.add)

                at = apool.tile([P, S], F32, tag="arow", name="at")
                for kg in range(ST // HC):
                    pt4 = ps_tr.tile([P, HC, P], BF16, tag="tr", name="pt4a")
                    for j in range(HC):
                        kt = kg * HC + j
                        nc.tensor.transpose(pt4[:, j, :], E[:, kt, esl],
                                            ident[:])
                    nc.scalar.mul(at[:, kg * 512:(kg + 1) * 512],
                                  pt4[:].rearrange("p a b -> p (a b)"),
                                  recips[:, qs:qs + 1])
                nc.sync.dma_start(attn_o[qt * P:(qt + 1) * P, :], at[:])
            ctx_v = ctx_o.rearrange("(t p) h -> p t h", p=P)
            nc.sync.dma_start(ctx_v[:, qb * 4:(qb + 1) * 4, :], cs4[:])

        emit_scores(0)
        for qb in range(NQB):
            if qb + 1 < NQB:
                emit_scores(qb + 1)
            emit_ctx_attn(qb)


def build_nc(fold):
    nc = bacc.Bacc("TRN2", target_bir_lowering=False, debug=False)
    X = nc.dram_tensor("X", [S, H], F32, kind="ExternalInput").ap()
    Wq = nc.dram_tensor("Wq", [H, H], F32, kind="ExternalInput").ap()
    Wk = nc.dram_tensor("Wk", [H, H], F32, kind="ExternalInput").ap()
    Wv = nc.dram_tensor("Wv", [H, H], F32, kind="ExternalInput").ap()
    bq_pc = nc.dram_tensor("bq_pc", [P, HC], F32, kind="ExternalInput").ap()
    bk_pc = nc.dram_tensor("bk_pc", [P, HC], F32, kind="ExternalInput").ap()
    bv_rep = nc.dram_tensor("bv_rep", [P, H], F32, kind="ExternalInput").ap()
    sr_rep = nc.dram_tensor("sr_rep", [P, 1], F32, kind="ExternalInput").ap()
    ctx_o = nc.dram_tensor("context", [S, H], F32, kind="ExternalOutput").ap()
    attn_o = nc.dram_tensor("attn", [S, S], F32, kind="ExternalOutput").ap()
    with tile.TileContext(nc) as tc:
        _emit(tc, nc, (X, Wq, Wk, Wv, bq_pc, bk_pc, bv_rep, sr_rep,
                       ctx_o, attn_o), fold)
    nc.compile()
    return nc


def get_nc(fold=True):
    key = "fold" if fold else "general"
    if key not in _CACHE:
        _CACHE[key] = build_nc(fold)
    return _CACHE[key]


def make_in_maps(inputs):
    X = np.ascontiguousarray(np.asarray(inputs["X"], dtype=np.float32))
    Wq = np.ascontiguousarray(np.asarray(inputs["Wq"], dtype=np.float32))
    Wk = np.ascontiguousarray(np.asarray(inputs["Wk"], dtype=np.float32))
    Wv = np.ascontiguousarray(np.asarray(inputs["Wv"], dtype=np.float32))
    bq = np.asarray(inputs["bq"], dtype=np.float32)
    bk = np.asarray(inputs["bk"], dtype=np.float32)
    bv = np.asarray(inputs["bv"], dtype=np.float32)
    sr = float(np.asarray(inputs["shape_ratio"]).reshape(-1)[0])
    # input marshalling: reshape/replicate only (no tensor math)
    bq_pc = np.ascontiguousarray(bq.reshape(HC, P).T)
    bk_pc = np.ascontiguousarray(bk.reshape(HC, P).T)
    bv_rep = np.ascontiguousarray(np.broadcast_to(bv.reshape(1, H), (P, H)))
    sr_rep = np.full((P, 1), sr, dtype=np.float32)
    fold = not (np.any(bq) or np.any(bk))
    return [
        dict(X=X[b], Wq=Wq, Wk=Wk, Wv=Wv, bq_pc=bq_pc, bk_pc=bk_pc,
             bv_rep=bv_rep, sr_rep=sr_rep)
        for b in range(B)
    ], fold


def kernel(**inputs):
    in_maps, fold = make_in_maps(inputs)
    nc = get_nc(fold)
    r = run_bass_kernel_spmd(nc, in_maps, list(range(B)))
    context = np.stack([r.results[b]["context"] for b in range(B)])
    attn = np.stack([r.results[b]["attn"] for b in range(B)])
    return context, attn
